# revision 9
# baseline (speedup 1.0000x reference)
"""CenterLoss on Trainium2, data-parallel across 8 NeuronCores.

reference:
    distmat = 0.5*(||x||^2 + ||c||^2) + 0.3 * x @ centers.T        [B, C]
    loss = sum(clip(distmat * onehot(labels), 1e-12, 1e12)) / B

The mask keeps exactly one entry per row (j == labels[i]); every other
entry becomes clip(0) = 1e-12.  So

    loss = ( sum_i clip(d_i, 1e-12, 1e12) + B*(C-1)*1e-12 ) / B
    d_i  = 0.5*(||x_i||^2 + ||c_{l_i}||^2) + 0.3 * x_i . c_{l_i}

Per core (512 rows): gather the 512 labeled center rows with DGE-ucode
dma_gather (4 instructions on 4 SWDGE queues, draining in parallel),
x rows loaded next to them so one Square+row-sum ACT pass per tile
covers 0.5*(||x||^2+||c||^2); one fused multiply+row-sum DVE pass gives
0.3*x.c; clip per row; a PE matmul against ones reduces partitions to a
scalar.  The host sums the 8 per-core scalars (the "all-reduce"), adds
the clip constant and divides by B.
"""

import os

import numpy as np

import concourse.bass as bass
import concourse.bacc as bacc
import concourse.tile as tile
from concourse import mybir
from concourse.bass_utils import run_bass_kernel_spmd

B, C, D = 4096, 10000, 512
NCORES = 8
BS = B // NCORES  # 512 rows per core
P = 128
NT = BS // P  # 4 partition-tiles per core

F32 = mybir.dt.float32
I16 = mybir.dt.int16

# Debug hooks for test.py (harness just calls kernel()).
TRACE = os.environ.get("KERNEL_TRACE", "0") == "1"
LAST_RESULTS = [None]

_NC_CACHE = []


def _build_bass():
    # Bacc (not raw Bass): its finalize() legalizes multi-wait instructions
    # into event semaphores, which walrus codegen requires.
    nc = bacc.Bacc(None, target_bir_lowering=False, num_swdge_queues=4)
    x_in = nc.dram_tensor("x", [BS, D], F32, kind="ExternalInput")
    # labels pre-wrapped on host into the DGE gather index layout:
    # element (p, s) = labels[s*16 + p%16], int16, replicated down all 128
    # partitions (8 copies of the 16-partition wrap, one per ucode core).
    lab_in = nc.dram_tensor("labels16", [P, BS // 16], I16, kind="ExternalInput")
    cen_in = nc.dram_tensor("centers", [C, D], F32, kind="ExternalInput")
    out_t = nc.dram_tensor("out", [1, 1], F32, kind="ExternalOutput")

    # x as [p, n, d]: row n*128+p of the shard
    x_pnd = x_in.rearrange("(n p) d -> p n d", p=P)

    with tile.TileContext(nc) as tc:
        with (
            tc.tile_pool(name="io", bufs=1) as io,
            tc.tile_pool(name="scratch", bufs=2) as scratch,
            tc.tile_pool(name="accp", bufs=1) as accp,
            tc.tile_pool(name="psum", bufs=1, space="PSUM") as psum,
        ):
            acc = accp.tile([P, NT], F32)
            ones = accp.tile([P, 1], F32)
            nc.vector.memset(ones[:], 1.0)

            # index tile first: the gathers depend on it
            idx16 = io.tile([P, BS // 16], I16)
            nc.sync.dma_start(out=idx16[:, :], in_=lab_in[:, :])

            # xc[:, k, 0:D] = x rows of tile k; xc[:, k, D:2D] = gathered centers
            xc = io.tile([P, NT, 2 * D], F32)
            for k in range(NT):
                nc.sync.dma_start(out=xc[:, k, 0:D], in_=x_pnd[:, k, :])
                # gather 128 center rows for tile k on its own SWDGE queue
                nc.gpsimd.dma_gather(
                    out_ap=xc[:, k : k + 1, D : 2 * D],
                    in_ap=cen_in[:, :],
                    idxs_ap=idx16[:, k * (P // 16) : (k + 1) * (P // 16)],
                    num_idxs=P,
                    num_idxs_reg=P,
                    elem_size=D,
                    queue_num=k,
                )

            for k in range(NT):
                # sq = sum(x^2) + sum(c^2) per row, one ACT pass over 2D
                sqf = scratch.tile([P, 2 * D], F32, tag="sqf")
                sq = scratch.tile([P, 1], F32, tag="sq")
                nc.scalar.activation(
                    out=sqf[:],
                    in_=xc[:, k, :],
                    func=mybir.ActivationFunctionType.Square,
                    accum_out=sq[:],
                )

                # dot = sum(0.3*x * c) per row
                prod = scratch.tile([P, D], F32, tag="prod")
                dot = scratch.tile([P, 1], F32, tag="dot")
                nc.vector.scalar_tensor_tensor(
                    out=prod[:],
                    in0=xc[:, k, 0:D],
                    scalar=0.3,
                    in1=xc[:, k, D : 2 * D],
                    op0=mybir.AluOpType.mult,
                    op1=mybir.AluOpType.mult,
                    accum_out=dot[:],
                )

                # d = 0.5*sq + dot
                d = scratch.tile([P, 1], F32, tag="d")
                nc.vector.scalar_tensor_tensor(
                    out=d[:],
                    in0=sq[:],
                    scalar=0.5,
                    in1=dot[:],
                    op0=mybir.AluOpType.mult,
                    op1=mybir.AluOpType.add,
                )

                # clip to [1e-12, 1e12], park in acc column k
                nc.vector.tensor_scalar(
                    out=acc[:, k : k + 1],
                    in0=d[:],
                    scalar1=1e-12,
                    scalar2=1e12,
                    op0=mybir.AluOpType.max,
                    op1=mybir.AluOpType.min,
                )

            accs = accp.tile([P, 1], F32)
            nc.vector.reduce_sum(out=accs[:], in_=acc[:], axis=mybir.AxisListType.X)

            # partition reduce: [1,128] @ [128,1] on PE
            ps = psum.tile([1, 1], F32)
            nc.tensor.matmul(out=ps[:], lhsT=accs[:], rhs=ones[:], start=True, stop=True)
            res = accp.tile([1, 1], F32)
            nc.vector.tensor_copy(out=res[:], in_=ps[:])
            nc.sync.dma_start(out=out_t[:, :], in_=res[:])
    nc.finalize()
    return nc


def _get_nc():
    if not _NC_CACHE:
        _NC_CACHE.append(_build_bass())
    return _NC_CACHE[0]


def _wrap_labels16(lab_shard):
    # DGE gather index layout: idx i lives at partition i % 16, column i // 16,
    # replicated down all 128 partitions (8 ucode cores read their own block)
    w = lab_shard.astype(np.int16).reshape(BS // 16, 16).T
    return np.ascontiguousarray(np.tile(w, (8, 1)))


def kernel(x, centers, labels):
    x = np.ascontiguousarray(np.asarray(x), dtype=np.float32)
    centers = np.ascontiguousarray(np.asarray(centers), dtype=np.float32)
    labels = np.asarray(labels).astype(np.int64)
    assert x.shape == (B, D) and centers.shape == (C, D) and labels.shape == (B,)

    nc = _get_nc()
    in_maps = [
        {
            "x": x[c * BS : (c + 1) * BS],
            "labels16": _wrap_labels16(labels[c * BS : (c + 1) * BS]),
            "centers": centers,
        }
        for c in range(NCORES)
    ]
    res = run_bass_kernel_spmd(nc, in_maps, core_ids=list(range(NCORES)), trace=TRACE)
    LAST_RESULTS[0] = res

    total = float(np.sum(np.array([r["out"][0, 0] for r in res.results], np.float64)))
    total += B * (C - 1) * 1e-12
    return np.array(total / B, dtype=np.float32)


# revision 17
# speedup vs baseline: 1.3240x; 1.3240x over previous
"""CenterLoss on Trainium2, data-parallel across 8 NeuronCores.

reference:
    distmat = 0.5*(||x||^2 + ||c||^2) + 0.3 * x @ centers.T        [B, C]
    loss = sum(clip(distmat * onehot(labels), 1e-12, 1e12)) / B

The mask keeps exactly one entry per row (j == labels[i]); every other
entry becomes clip(0) = 1e-12.  So

    loss = ( sum_i clip(d_i, 1e-12, 1e12) + B*(C-1)*1e-12 ) / B
    d_i  = 0.5*(||x_i||^2 + ||c_{l_i}||^2) + 0.3 * x_i . c_{l_i}

Per core (512 rows):
  - labels arrive via one tiny DMA on the ACT HWDGE ring (lowest-latency
    ring with no other traffic), then 4 indirect-DMA gathers fetch the
    labeled center rows tile by tile (FIFO, so tile 0 lands first);
  - x rows stream in on the SP HWDGE ring; ||x||^2 row-sums run on the
    Vector engine while the gathers are still in flight;
  - per gathered tile: ||c||^2 on ACT (Square+accum), 0.3*x.c on Vector
    (fused multiply+row-sum), then d = 0.5*(sqx+sqc) + dot, clipped;
  - a PE matmul against ones reduces partitions to the per-core scalar.
The host sums the 8 per-core scalars (the "all-reduce"), adds the clip
constant and divides by B.
"""

import os

import numpy as np

import concourse.bass as bass
import concourse.bacc as bacc
import concourse.tile as tile
from concourse import mybir
from concourse.bass_utils import run_bass_kernel_spmd

B, C, D = 4096, 10000, 512
NCORES = 8
BS = B // NCORES  # 512 rows per core
P = 128
NT = BS // P  # 4 partition-tiles per core

F32 = mybir.dt.float32
I32 = mybir.dt.int32

# Debug hooks for test.py (harness just calls kernel()).
TRACE = os.environ.get("KERNEL_TRACE", "0") == "1"
LAST_RESULTS = [None]

_NC_CACHE = []


def _build_bass():
    # Bacc (not raw Bass): its finalize() legalizes multi-wait instructions
    # into event semaphores, which walrus codegen requires.
    nc = bacc.Bacc(None, target_bir_lowering=False, num_swdge_queues=4)
    x_in = nc.dram_tensor("x", [BS, D], F32, kind="ExternalInput")
    lab_in = nc.dram_tensor("labels", [BS], I32, kind="ExternalInput")
    cen_in = nc.dram_tensor("centers", [C, D], F32, kind="ExternalInput")
    out_t = nc.dram_tensor("out", [1, 1], F32, kind="ExternalOutput")

    # shard row n*128+p lives at partition p, slot n
    x_pnd = x_in.rearrange("(n p) d -> p n d", p=P)
    lab_pn = lab_in.rearrange("(n p) -> p n", p=P)

    with tile.TileContext(nc) as tc:
        with (
            tc.tile_pool(name="io", bufs=1) as io,
            tc.tile_pool(name="scratch", bufs=2) as scratch,
            tc.tile_pool(name="accp", bufs=1) as accp,
            tc.tile_pool(name="psum", bufs=1, space="PSUM") as psum,
        ):
            acc = accp.tile([P, NT], F32)
            ones = accp.tile([P, 1], F32)
            nc.vector.memset(ones[:], 1.0)

            # all 4 index columns in one tiny DMA on the otherwise-idle ACT
            # HWDGE ring, issued first: the gathers depend on it
            idx = io.tile([P, NT], I32)
            nc.scalar.dma_start(out=idx[:, :], in_=lab_pn[:, :])

            cs = [io.tile([P, D], F32, name=f"c{k}", tag=f"c{k}") for k in range(NT)]
            xs = [io.tile([P, D], F32, name=f"x{k}", tag=f"x{k}") for k in range(NT)]
            for k in range(NT):
                # FIFO on the SWDGE queue: tile k's rows complete before k+1's
                gi = nc.gpsimd.indirect_dma_start(
                    out=cs[k][:, :],
                    out_offset=None,
                    in_=cen_in[:],
                    in_offset=bass.IndirectOffsetOnAxis(ap=idx[:, k : k + 1], axis=0),
                )
                gi.ins.queue = f"qPoolDynamic{k or ''}"
            for k in range(NT):
                nc.sync.dma_start(out=xs[k][:, :], in_=x_pnd[:, k, :])

            # ||x||^2 on Vector, hidden under the gather wait
            sqxs = []
            for k in range(NT):
                xx = scratch.tile([P, D], F32, tag="xx")
                sqx = accp.tile([P, 1], F32, name=f"sqx{k}", tag=f"sqx{k}")
                nc.vector.scalar_tensor_tensor(
                    out=xx[:],
                    in0=xs[k][:, :],
                    scalar=1.0,
                    in1=xs[k][:, :],
                    op0=mybir.AluOpType.mult,
                    op1=mybir.AluOpType.mult,
                    accum_out=sqx[:],
                )
                sqxs.append(sqx)

            for k in range(NT):
                # sqc = sum(c^2) per row on ACT
                cc = scratch.tile([P, D], F32, tag="cc")
                sqc = scratch.tile([P, 1], F32, tag="sqc")
                nc.scalar.activation(
                    out=cc[:],
                    in_=cs[k][:, :],
                    func=mybir.ActivationFunctionType.Square,
                    accum_out=sqc[:],
                )

                # dot = sum(0.3*x * c) per row on Vector
                prod = scratch.tile([P, D], F32, tag="prod")
                dot = scratch.tile([P, 1], F32, tag="dot")
                nc.vector.scalar_tensor_tensor(
                    out=prod[:],
                    in0=xs[k][:, :],
                    scalar=0.3,
                    in1=cs[k][:, :],
                    op0=mybir.AluOpType.mult,
                    op1=mybir.AluOpType.mult,
                    accum_out=dot[:],
                )

                # d = 0.5*(sqx + sqc) + dot
                u = scratch.tile([P, 1], F32, tag="u")
                nc.vector.tensor_add(out=u[:], in0=sqxs[k][:], in1=sqc[:])
                d = scratch.tile([P, 1], F32, tag="d")
                nc.vector.scalar_tensor_tensor(
                    out=d[:],
                    in0=u[:],
                    scalar=0.5,
                    in1=dot[:],
                    op0=mybir.AluOpType.mult,
                    op1=mybir.AluOpType.add,
                )

                # clip to [1e-12, 1e12], park in acc column k
                nc.vector.tensor_scalar(
                    out=acc[:, k : k + 1],
                    in0=d[:],
                    scalar1=1e-12,
                    scalar2=1e12,
                    op0=mybir.AluOpType.max,
                    op1=mybir.AluOpType.min,
                )

            accs = accp.tile([P, 1], F32)
            nc.vector.reduce_sum(out=accs[:], in_=acc[:], axis=mybir.AxisListType.X)

            # partition reduce: [1,128] @ [128,1] on PE
            ps = psum.tile([1, 1], F32)
            nc.tensor.matmul(out=ps[:], lhsT=accs[:], rhs=ones[:], start=True, stop=True)
            res = accp.tile([1, 1], F32)
            nc.vector.tensor_copy(out=res[:], in_=ps[:])
            nc.sync.dma_start(out=out_t[:, :], in_=res[:])
    nc.finalize()
    return nc


def _get_nc():
    if not _NC_CACHE:
        _NC_CACHE.append(_build_bass())
    return _NC_CACHE[0]


def kernel(x, centers, labels):
    x = np.ascontiguousarray(np.asarray(x), dtype=np.float32)
    centers = np.ascontiguousarray(np.asarray(centers), dtype=np.float32)
    labels = np.ascontiguousarray(np.asarray(labels).astype(np.int32))
    assert x.shape == (B, D) and centers.shape == (C, D) and labels.shape == (B,)

    nc = _get_nc()
    in_maps = [
        {
            "x": x[c * BS : (c + 1) * BS],
            "labels": labels[c * BS : (c + 1) * BS],
            "centers": centers,
        }
        for c in range(NCORES)
    ]
    res = run_bass_kernel_spmd(nc, in_maps, core_ids=list(range(NCORES)), trace=TRACE)
    LAST_RESULTS[0] = res

    total = float(np.sum(np.array([r["out"][0, 0] for r in res.results], np.float64)))
    total += B * (C - 1) * 1e-12
    return np.array(total / B, dtype=np.float32)


# revision 19
# speedup vs baseline: 1.3634x; 1.0297x over previous
"""CenterLoss on Trainium2, data-parallel across 8 NeuronCores.

reference:
    distmat = 0.5*(||x||^2 + ||c||^2) + 0.3 * x @ centers.T        [B, C]
    loss = sum(clip(distmat * onehot(labels), 1e-12, 1e12)) / B

The mask keeps exactly one entry per row (j == labels[i]); every other
entry becomes clip(0) = 1e-12.  So

    loss = ( sum_i clip(d_i, 1e-12, 1e12) + B*(C-1)*1e-12 ) / B
    d_i  = 0.5*(||x_i||^2 + ||c_{l_i}||^2) + 0.3 * x_i . c_{l_i}

Per core (512 rows):
  - labels arrive via one tiny DMA on the ACT HWDGE ring (lowest-latency
    ring with no other traffic), then 4 indirect-DMA gathers fetch the
    labeled center rows tile by tile (FIFO, so tile 0 lands first);
  - x rows stream in on the SP HWDGE ring; ||x||^2 row-sums run on the
    Vector engine while the gathers are still in flight;
  - per gathered tile: ||c||^2 on ACT (Square+accum), 0.3*x.c on Vector
    (fused multiply+row-sum), then d = 0.5*(sqx+sqc) + dot, clipped;
  - a PE matmul against ones reduces partitions to the per-core scalar.
The host sums the 8 per-core scalars (the "all-reduce"), adds the clip
constant and divides by B.
"""

import os

import numpy as np

import concourse.bass as bass
import concourse.bacc as bacc
import concourse.tile as tile
from concourse import mybir
from concourse.bass_utils import run_bass_kernel_spmd

B, C, D = 4096, 10000, 512
NCORES = 8
BS = B // NCORES  # 512 rows per core
P = 128
NT = BS // P  # 4 partition-tiles per core

F32 = mybir.dt.float32
I32 = mybir.dt.int32

# Debug hooks for test.py (harness just calls kernel()).
TRACE = os.environ.get("KERNEL_TRACE", "0") == "1"
LAST_RESULTS = [None]

_NC_CACHE = []


def _build_bass():
    # Bacc (not raw Bass): its finalize() legalizes multi-wait instructions
    # into event semaphores, which walrus codegen requires.
    nc = bacc.Bacc(None, target_bir_lowering=False, num_swdge_queues=4)
    x_in = nc.dram_tensor("x", [BS, D], F32, kind="ExternalInput")
    lab_in = nc.dram_tensor("labels", [BS], I32, kind="ExternalInput")
    cen_in = nc.dram_tensor("centers", [C, D], F32, kind="ExternalInput")
    out_t = nc.dram_tensor("out", [1, 1], F32, kind="ExternalOutput")

    # shard row n*128+p lives at partition p, slot n
    x_pnd = x_in.rearrange("(n p) d -> p n d", p=P)
    lab_pn = lab_in.rearrange("(n p) -> p n", p=P)

    with tile.TileContext(nc) as tc:
        with (
            tc.tile_pool(name="io", bufs=1) as io,
            tc.tile_pool(name="scratch", bufs=2) as scratch,
            tc.tile_pool(name="accp", bufs=1) as accp,
            tc.tile_pool(name="psum", bufs=1, space="PSUM") as psum,
        ):
            acc = accp.tile([P, NT], F32)
            ones = accp.tile([P, 1], F32)
            nc.vector.memset(ones[:], 1.0)

            # all 4 index columns in one tiny HWDGE DMA, issued first: the
            # gathers depend on it (measured: SP ring beats the ACT ring here,
            # the ACT ring start is delayed by the activation table load)
            idx = io.tile([P, NT], I32)
            nc.sync.dma_start(out=idx[:, :], in_=lab_pn[:, :])

            cs = [io.tile([P, D], F32, name=f"c{k}", tag=f"c{k}") for k in range(NT)]
            xs = [io.tile([P, D], F32, name=f"x{k}", tag=f"x{k}") for k in range(NT)]
            for k in range(NT):
                # FIFO on the SWDGE queue: tile k's rows complete before k+1's
                gi = nc.gpsimd.indirect_dma_start(
                    out=cs[k][:, :],
                    out_offset=None,
                    in_=cen_in[:],
                    in_offset=bass.IndirectOffsetOnAxis(ap=idx[:, k : k + 1], axis=0),
                )
                gi.ins.queue = f"qPoolDynamic{k or ''}"
            for k in range(NT):
                nc.sync.dma_start(out=xs[k][:, :], in_=x_pnd[:, k, :])

            # per-row partial sums land in columns of [P, NT] tiles so the
            # final combine runs as a few [P, NT]-wide ops instead of a chain
            # of 4x per-tile scalar ops
            sqx_all = accp.tile([P, NT], F32)
            sqc_all = accp.tile([P, NT], F32)
            dot_all = accp.tile([P, NT], F32)

            # ||x||^2 on Vector, hidden under the gather wait
            for k in range(NT):
                xx = scratch.tile([P, D], F32, tag="xx")
                nc.vector.scalar_tensor_tensor(
                    out=xx[:],
                    in0=xs[k][:, :],
                    scalar=1.0,
                    in1=xs[k][:, :],
                    op0=mybir.AluOpType.mult,
                    op1=mybir.AluOpType.mult,
                    accum_out=sqx_all[:, k : k + 1],
                )

            for k in range(NT):
                # sqc = sum(c^2) per row on ACT
                cc = scratch.tile([P, D], F32, tag="cc")
                nc.scalar.activation(
                    out=cc[:],
                    in_=cs[k][:, :],
                    func=mybir.ActivationFunctionType.Square,
                    accum_out=sqc_all[:, k : k + 1],
                )

                # dot = sum(0.3*x * c) per row on Vector
                prod = scratch.tile([P, D], F32, tag="prod")
                nc.vector.scalar_tensor_tensor(
                    out=prod[:],
                    in0=xs[k][:, :],
                    scalar=0.3,
                    in1=cs[k][:, :],
                    op0=mybir.AluOpType.mult,
                    op1=mybir.AluOpType.mult,
                    accum_out=dot_all[:, k : k + 1],
                )

            # d = 0.5*(sqx + sqc) + dot for all tiles at once, then clip
            u = accp.tile([P, NT], F32)
            nc.vector.tensor_add(out=u[:], in0=sqx_all[:], in1=sqc_all[:])
            dall = accp.tile([P, NT], F32)
            nc.vector.scalar_tensor_tensor(
                out=dall[:],
                in0=u[:],
                scalar=0.5,
                in1=dot_all[:],
                op0=mybir.AluOpType.mult,
                op1=mybir.AluOpType.add,
            )
            nc.vector.tensor_scalar(
                out=acc[:, :],
                in0=dall[:],
                scalar1=1e-12,
                scalar2=1e12,
                op0=mybir.AluOpType.max,
                op1=mybir.AluOpType.min,
            )

            accs = accp.tile([P, 1], F32)
            nc.vector.reduce_sum(out=accs[:], in_=acc[:], axis=mybir.AxisListType.X)

            # partition reduce: [1,128] @ [128,1] on PE
            ps = psum.tile([1, 1], F32)
            nc.tensor.matmul(out=ps[:], lhsT=accs[:], rhs=ones[:], start=True, stop=True)
            res = accp.tile([1, 1], F32)
            nc.vector.tensor_copy(out=res[:], in_=ps[:])
            nc.sync.dma_start(out=out_t[:, :], in_=res[:])
    nc.finalize()
    return nc


def _get_nc():
    if not _NC_CACHE:
        _NC_CACHE.append(_build_bass())
    return _NC_CACHE[0]


def kernel(x, centers, labels):
    x = np.ascontiguousarray(np.asarray(x), dtype=np.float32)
    centers = np.ascontiguousarray(np.asarray(centers), dtype=np.float32)
    labels = np.ascontiguousarray(np.asarray(labels).astype(np.int32))
    assert x.shape == (B, D) and centers.shape == (C, D) and labels.shape == (B,)

    nc = _get_nc()
    in_maps = [
        {
            "x": x[c * BS : (c + 1) * BS],
            "labels": labels[c * BS : (c + 1) * BS],
            "centers": centers,
        }
        for c in range(NCORES)
    ]
    res = run_bass_kernel_spmd(nc, in_maps, core_ids=list(range(NCORES)), trace=TRACE)
    LAST_RESULTS[0] = res

    total = float(np.sum(np.array([r["out"][0, 0] for r in res.results], np.float64)))
    total += B * (C - 1) * 1e-12
    return np.array(total / B, dtype=np.float32)
